# revision 14
# baseline (speedup 1.0000x reference)
"""Trainium2 Bass kernel for quantized linear: out = (x @ w.T + bias) * scale.

Shapes (hardcoded): x[16384,1024] i32 (int8-range), w[4096,1024] i32 (int8-range),
scale[4096] f32, bias[4096] i32  ->  out[16384,4096] f32.

Strategy:
- Shard M (rows of x) across 8 cores: each core computes out[c*2048:(c+1)*2048, :].
- int8-range values are exact in bf16, and every partial sum of the i32 matmul
  is an integer of magnitude <= 1024*128*128 = 2^24, exactly representable in
  fp32. So a bf16 matmul with fp32 PSUM accumulation is bit-exact.
- Compute out.T per core (lhsT = w.T tile, rhs = x.T tile) so the per-out-channel
  scale/bias land on PSUM partitions: dequant is one ScalarE/VectorE affine op
  per tile.

Schedule (v8). PE roofline is 218.45us/core; the measured overheads are a
fixed ~7.8us NEFF engine preamble, the startup x-delivery window (chip-HBM
bound: all 8 cores stream x at once, so x0 cannot land before ~11-12us), and
a ~5.5us drain tail (HBM store receipt + framework barrier). Mid-kernel
periodic 432ns "gaps" in traces are dropped trace records, not stalls.
- All input loads ride the sync HWDGE ring as few, BIG DMAs in need order
  (w0, x0, w1, x1..x7, consts, w2, w3): the scalar ring is ~2x slower during
  the crunch and ANY concurrent early traffic delays x 1:1.
- 10 dummy matmuls (no DMA deps) bridge the preamble -> x0-arrival window and
  warm the HAM clock gate (PE runs 1.2GHz until ~3.4us sustained activity).
- nt=0,1 run "pair-k-outer" across all 8 PSUM banks: 8 MMs per x k-slice =
  1.7us/slice warm consumption vs ~1.55us/slice crunch-limited delivery, so
  the early x stalls of a serial-nt schedule (~2us) are absorbed.
- nt=2 chunk-outer (bank demand 1/1.7us matches the pair drain rate);
  nt>=3 k-outer with weight prefetch distance 2 on sync.
- Dequant alternates ScalarE/VectorE (halves bank-eviction latency); ONE
  batched 1MB store per nt on sync (4 per-chunk stores on ScalarE cost
  ~600ns sequencer each and made ScalarE the tail bottleneck).
- Tail: nt=31 chunk-outer with narrowing chunks (512,512,512,256,128,128),
  per-bank dequant+store as soon as each chain stops, stores spread across
  both HWDGE rings; nt=30 stores in halves so its 1MB is not the kernel's
  last DMA completion.
"""

import os

import numpy as np
import ml_dtypes

M, K, N = 16384, 1024, 4096
NCORES = 8
MS = M // NCORES  # 2048 rows of x per core
P = 128
KO = K // P  # 8 k-tiles
NT = N // P  # 32 n-tiles (PSUM partition dim = out-channel)
MC = 512  # psum free dim (one bank of fp32)
NMC = MS // MC  # 4 m-chunks per core

_CACHE = {}
LAST_RESULTS = None  # stash of BassKernelResults for test harnesses


def _build():
    import concourse.mybir as mybir
    import concourse.tile as tile
    from concourse import bacc

    dt = mybir.dt
    nc = bacc.Bacc("TRN2", target_bir_lowering=False, debug=False, num_devices=NCORES)

    # Host-pretiled layouts (see kernel() below):
    #   xT[p, ko, m]      = x_shard[m, ko*128+p]          (bf16)
    #   wt[nt, p, ko, nl] = w[nt*128+nl, ko*128+p]        (bf16)
    #   sc[p, nt]         = scale[nt*128+p]               (f32)
    #   bi[p, nt]         = scale[nt*128+p]*bias[nt*128+p](f32)
    #   outT[n, m]        = out_shard[m, n]               (f32)
    xT = nc.dram_tensor("xT", [P, KO, MS], dt.bfloat16, kind="ExternalInput").ap()
    wt = nc.dram_tensor("wt", [NT, P, KO, P], dt.bfloat16, kind="ExternalInput").ap()
    sc = nc.dram_tensor("sc", [P, NT], dt.float32, kind="ExternalInput").ap()
    bi = nc.dram_tensor("bi", [P, NT], dt.float32, kind="ExternalInput").ap()
    outT = nc.dram_tensor("outT", [N, MS], dt.float32, kind="ExternalOutput").ap()
    outT_t = outT.rearrange("(nt p) m -> nt p m", p=P)

    with tile.TileContext(nc) as tc:
        with (
            tc.tile_pool(name="xpool", bufs=1) as xpool,
            tc.tile_pool(name="wpool", bufs=6) as wpool,
            tc.tile_pool(name="cpool", bufs=1) as cpool,
            tc.tile_pool(name="opool", bufs=3) as opool,
            tc.tile_pool(name="ofpool", bufs=8) as ofpool,
            tc.tile_pool(name="psum", bufs=8, space="PSUM") as psum_pool,
        ):
            sc_sb = None
            bi_sb = None

            def dequant(eng, ot, ps, nt):
                if eng == "s":
                    nc.scalar.activation(
                        ot,
                        ps,
                        mybir.ActivationFunctionType.Identity,
                        bias=bi_sb[:, nt : nt + 1],
                        scale=sc_sb[:, nt : nt + 1],
                    )
                else:
                    nc.vector.tensor_scalar(
                        ot,
                        ps,
                        sc_sb[:, nt : nt + 1],
                        bi_sb[:, nt : nt + 1],
                        mybir.AluOpType.mult,
                        mybir.AluOpType.add,
                    )

            # ---- DVE first instruction: memset for the warm-up tile.
            warm = cpool.tile([P, MC], dt.bfloat16)
            nc.vector.memset(warm[:], 0.0)

            # ---- startup DMA program: few, big loads, all on the sync ring
            # in need order (the scalar ring is measurably ~2x slower during
            # the 8-core startup crunch; many small early DMAs also collapse
            # ring throughput). Order: w0, x0, w1, then the rest of x.
            w_tiles = {}

            def load_w(eng, nt):
                t = wpool.tile([P, KO, P], dt.bfloat16, tag="w", name=f"w_{nt}")
                eng.dma_start(t[:], wt[nt])
                w_tiles[nt] = t

            load_w(nc.sync, 0)
            x0 = xpool.tile([P, MS], dt.bfloat16, tag="x0", name="x_0")
            nc.sync.dma_start(x0[:], xT[:, 0])
            load_w(nc.sync, 1)
            x_ko = {0: x0}
            for ko in range(1, KO):
                t = xpool.tile([P, MS], dt.bfloat16, tag=f"x{ko}", name=f"x_{ko}")
                nc.sync.dma_start(t[:], xT[:, ko])
                x_ko[ko] = t

            # constants + w2/w3 on sync AFTER the x stream: any concurrent
            # traffic on other rings during the startup crunch delays x and
            # can re-throttle the HAM clock (measured: +1.9us on x1).
            sc_sb = cpool.tile([P, NT], dt.float32)
            nc.sync.dma_start(sc_sb[:], sc)
            bi_sb = cpool.tile([P, NT], dt.float32)
            nc.sync.dma_start(bi_sb[:], bi)
            load_w(nc.sync, 2)
            load_w(nc.sync, 3)

            # ---- warm-up dummies (no DMA deps): PE is HAM-throttled to
            # 1.2 GHz until ~3.4us of sustained activity.
            warm_ps = psum_pool.tile([P, MC], dt.float32, tag="ps", name="warm_ps")
            for _ in range(9):
                nc.tensor.matmul(
                    warm_ps[:], lhsT=warm[:, :P], rhs=warm[:], start=True, stop=True
                )

            # ---- pair phase: nt=0 and nt=1 together, k-outer across all 8
            # PSUM banks: 8 MMs per x k-slice = 1.7us (warm) per 512KB.
            pair_ps = {}
            for nt in (0, 1):
                for ci in range(NMC):
                    pair_ps[(nt, ci)] = psum_pool.tile(
                        [P, MC], dt.float32, tag="ps", name=f"ps_p{nt}_{ci}"
                    )
            for k in range(KO):
                for nt in (0, 1):
                    for ci in range(NMC):
                        off = ci * MC
                        rhs = x_ko[k][:, off : off + MC]
                        lhsT = w_tiles[nt][:, k]
                        nc.tensor.matmul(
                            pair_ps[(nt, ci)][:],
                            lhsT=lhsT,
                            rhs=rhs,
                            start=(k == 0),
                            stop=(k == KO - 1),
                        )
            obig = {}
            for nt in (0, 1):
                obig[nt] = opool.tile([P, MS], dt.float32, tag="o", name=f"o_{nt}")
                for ci in range(NMC):
                    dequant(
                        "s" if ci % 2 == 0 else "v",
                        obig[nt][:, ci * MC : (ci + 1) * MC],
                        pair_ps[(nt, ci)][:],
                        nt,
                    )
            del w_tiles[0], w_tiles[1]
            nc.sync.dma_start(outT_t[0], obig[0][:])
            nc.scalar.dma_start(outT_t[1], obig[1][:])

            def x_rhs(k, off, wd):
                return x_ko[k][:, off : off + wd]

            # ---- nt=2: chunk-outer (bank demand 1 per 1.7us matches the
            # drain rate of the pair phase's 8 banks).
            w_sb = w_tiles.pop(2)
            load_w(nc.sync, 4)
            o2 = opool.tile([P, MS], dt.float32, tag="o", name="o_2")
            for ci in range(NMC):
                ps = psum_pool.tile([P, MC], dt.float32, tag="ps", name=f"ps_2_{ci}")
                for k in range(KO):
                    nc.tensor.matmul(
                        ps[:],
                        lhsT=w_sb[:, k],
                        rhs=x_rhs(k, ci * MC, MC),
                        start=(k == 0),
                        stop=(k == KO - 1),
                    )
                dequant(
                    "s" if ci % 2 == 0 else "v",
                    o2[:, ci * MC : (ci + 1) * MC],
                    ps[:],
                    2,
                )
            nc.sync.dma_start(outT_t[2], o2[:])

            # ---- steady state: nt=3..30, k-outer, weight prefetch distance 2
            # on sync, one batched store per nt on sync.
            for nt in range(3, NT - 1):
                if nt + 2 < NT:
                    load_w(nc.sync, nt + 2)
                w_sb = w_tiles.pop(nt)
                psums = [
                    psum_pool.tile([P, MC], dt.float32, tag="ps", name=f"ps_{nt}_{ci}")
                    for ci in range(NMC)
                ]
                for k in range(KO):
                    for ci in range(NMC):
                        nc.tensor.matmul(
                            psums[ci][:],
                            lhsT=w_sb[:, k],
                            rhs=x_rhs(k, ci * MC, MC),
                            start=(k == 0),
                            stop=(k == KO - 1),
                        )
                ot = opool.tile([P, MS], dt.float32, tag="o", name=f"o_{nt}")
                for ci in range(NMC):
                    dequant(
                        "s" if ci % 2 == 0 else "v",
                        ot[:, ci * MC : (ci + 1) * MC],
                        psums[ci][:],
                        nt,
                    )
                if nt < NT - 2:
                    nc.sync.dma_start(outT_t[nt], ot[:])
                else:
                    # nt=30: store in halves so its 1MB isn't the kernel's
                    # last DMA completion.
                    nc.sync.dma_start(outT_t[nt, :, :1024], ot[:, :1024])
                    nc.sync.dma_start(outT_t[nt, :, 1024:], ot[:, 1024:])

            # ---- tail: nt=31 chunk-outer with narrowing chunks; each bank
            # dequants+stores as soon as its own chain stops; stores spread
            # across both HWDGE rings.
            nt = NT - 1
            w_sb = w_tiles.pop(nt)
            chunks = [
                (0, 512),
                (512, 512),
                (1024, 512),
                (1536, 256),
                (1792, 128),
                (1920, 64),
                (1984, 64),
            ]
            for ci, (off, wd) in enumerate(chunks):
                ps = psum_pool.tile([P, wd], dt.float32, tag="ps", name=f"ps_t_{ci}")
                for k in range(KO):
                    nc.tensor.matmul(
                        ps[:],
                        lhsT=w_sb[:, k],
                        rhs=x_rhs(k, off, wd),
                        start=(k == 0),
                        stop=(k == KO - 1),
                    )
                ot = ofpool.tile([P, MC], dt.float32, tag="of", name=f"of_{ci}")
                ot = ot[:, :wd]
                dequant("s" if ci % 2 == 0 else "v", ot, ps[:], nt)
                if ci % 2 == 1:
                    nc.sync.dma_start(outT_t[nt, :, off : off + wd], ot)
                else:
                    nc.scalar.dma_start(outT_t[nt, :, off : off + wd], ot)

    nc.compile()
    return nc


def _get_nc():
    if "nc" not in _CACHE:
        _CACHE["nc"] = _build()
    return _CACHE["nc"]


def _try_install_ntff_hook():
    """Best-effort: register the axon NTFF profiling hook (the agent image's
    antenv lacks axon_hooks). Returns True if tracing is usable."""
    try:
        import sys
        import types

        import antenv

        if "antenv.axon_hooks" not in sys.modules:
            mod = types.ModuleType("antenv.axon_hooks")
            state = {"hook": None}
            mod.set_axon_ntff_profile_hook = lambda h: state.__setitem__("hook", h)
            mod.get_axon_ntff_profile_hook = lambda: state["hook"]
            sys.modules["antenv.axon_hooks"] = mod
            antenv.axon_hooks = mod

            from trn_agent_boot.trn_boot import _ntff_profile_via_ctypes

            hook = _ntff_profile_via_ctypes("/opt/axon/libaxon_pjrt.so")
            if hook is not None:
                mod.set_axon_ntff_profile_hook(hook)
        return True
    except Exception:
        return False


def kernel(**inputs) -> np.ndarray:
    global LAST_RESULTS
    from concourse.bass_utils import run_bass_kernel_spmd

    x = np.asarray(inputs["x"])
    w = np.asarray(inputs["weight"])
    scale = np.asarray(inputs["scale"], dtype=np.float32)
    bias = np.asarray(inputs["bias"])

    bf16 = ml_dtypes.bfloat16
    nc = _get_nc()

    # weight -> [nt, k_local(part), ko, n_local]
    wt = np.ascontiguousarray(
        w.astype(bf16).reshape(NT, P, KO, P).transpose(0, 3, 2, 1)
    )
    sc = np.ascontiguousarray(scale.reshape(NT, P).T)
    bi = np.ascontiguousarray((bias.astype(np.float32) * scale).reshape(NT, P).T)

    in_maps = []
    for c in range(NCORES):
        xs = x[c * MS : (c + 1) * MS].astype(bf16)  # [MS, K]
        xt = np.ascontiguousarray(xs.T.reshape(KO, P, MS).transpose(1, 0, 2))
        in_maps.append({"xT": xt, "wt": wt, "sc": sc, "bi": bi})

    trace = os.environ.get("BASS_TRACE", "0") == "1" and _try_install_ntff_hook()
    try:
        LAST_RESULTS = run_bass_kernel_spmd(
            nc, in_maps, core_ids=list(range(NCORES)), trace=trace
        )
    except Exception:
        if not trace:
            raise
        # Tracing plumbing is environment-dependent; never let it take down
        # the actual computation.
        os.environ["BASS_NEVER_TRACE"] = "1"
        LAST_RESULTS = run_bass_kernel_spmd(
            nc, in_maps, core_ids=list(range(NCORES)), trace=False
        )

    out = np.empty((M, N), dtype=np.float32)
    for c in range(NCORES):
        out[c * MS : (c + 1) * MS] = LAST_RESULTS.results[c]["outT"].T
    return out


# revision 15
# speedup vs baseline: 1.0078x; 1.0078x over previous
"""Trainium2 Bass kernel for quantized linear: out = (x @ w.T + bias) * scale.

Shapes (hardcoded): x[16384,1024] i32 (int8-range), w[4096,1024] i32 (int8-range),
scale[4096] f32, bias[4096] i32  ->  out[16384,4096] f32.

Strategy:
- Shard M (rows of x) across 8 cores: each core computes out[c*2048:(c+1)*2048, :].
- int8-range values are exact in bf16, and every partial sum of the i32 matmul
  is an integer of magnitude <= 1024*128*128 = 2^24, exactly representable in
  fp32. So a bf16 matmul with fp32 PSUM accumulation is bit-exact.
- Compute out.T per core (lhsT = w.T tile, rhs = x.T tile) so the per-out-channel
  scale/bias land on PSUM partitions: dequant is one ScalarE/VectorE affine op
  per tile.

Schedule (v8). PE roofline is 218.45us/core; the measured overheads are a
fixed ~7.8us NEFF engine preamble, the startup x-delivery window (chip-HBM
bound: all 8 cores stream x at once, so x0 cannot land before ~11-12us), and
a ~5.5us drain tail (HBM store receipt + framework barrier). Mid-kernel
periodic 432ns "gaps" in traces are dropped trace records, not stalls.
- All input loads ride the sync HWDGE ring as few, BIG DMAs in need order
  (w0, x0, w1, x1..x7, consts, w2, w3): the scalar ring is ~2x slower during
  the crunch and ANY concurrent early traffic delays x 1:1.
- 10 dummy matmuls (no DMA deps) bridge the preamble -> x0-arrival window and
  warm the HAM clock gate (PE runs 1.2GHz until ~3.4us sustained activity).
- nt=0,1 run "pair-k-outer" across all 8 PSUM banks: 8 MMs per x k-slice =
  1.7us/slice warm consumption vs ~1.55us/slice crunch-limited delivery, so
  the early x stalls of a serial-nt schedule (~2us) are absorbed.
- nt=2 chunk-outer (bank demand 1/1.7us matches the pair drain rate);
  nt>=3 k-outer with weight prefetch distance 2 on sync.
- Dequant alternates ScalarE/VectorE (halves bank-eviction latency); ONE
  batched 1MB store per nt on sync (4 per-chunk stores on ScalarE cost
  ~600ns sequencer each and made ScalarE the tail bottleneck).
- Tail: nt=31 chunk-outer with narrowing chunks (512,512,512,256,128,128),
  per-bank dequant+store as soon as each chain stops, stores spread across
  both HWDGE rings; nt=30 stores in halves so its 1MB is not the kernel's
  last DMA completion.
"""

import os

import numpy as np
import ml_dtypes

M, K, N = 16384, 1024, 4096
NCORES = 8
MS = M // NCORES  # 2048 rows of x per core
P = 128
KO = K // P  # 8 k-tiles
NT = N // P  # 32 n-tiles (PSUM partition dim = out-channel)
MC = 512  # psum free dim (one bank of fp32)
NMC = MS // MC  # 4 m-chunks per core

_CACHE = {}
LAST_RESULTS = None  # stash of BassKernelResults for test harnesses


def _build():
    import concourse.mybir as mybir
    import concourse.tile as tile
    from concourse import bacc

    dt = mybir.dt
    nc = bacc.Bacc("TRN2", target_bir_lowering=False, debug=False, num_devices=NCORES)

    # Host-pretiled layouts (see kernel() below):
    #   xT[p, ko, m]      = x_shard[m, ko*128+p]          (bf16)
    #   wt[nt, p, ko, nl] = w[nt*128+nl, ko*128+p]        (bf16)
    #   sc[p, nt]         = scale[nt*128+p]               (f32)
    #   bi[p, nt]         = scale[nt*128+p]*bias[nt*128+p](f32)
    #   outT[n, m]        = out_shard[m, n]               (f32)
    xT = nc.dram_tensor("xT", [P, KO, MS], dt.bfloat16, kind="ExternalInput").ap()
    wt = nc.dram_tensor("wt", [NT, P, KO, P], dt.bfloat16, kind="ExternalInput").ap()
    sc = nc.dram_tensor("sc", [P, NT], dt.float32, kind="ExternalInput").ap()
    bi = nc.dram_tensor("bi", [P, NT], dt.float32, kind="ExternalInput").ap()
    outT = nc.dram_tensor("outT", [N, MS], dt.float32, kind="ExternalOutput").ap()
    outT_t = outT.rearrange("(nt p) m -> nt p m", p=P)

    with tile.TileContext(nc) as tc:
        with (
            tc.tile_pool(name="xpool", bufs=1) as xpool,
            tc.tile_pool(name="wpool", bufs=6) as wpool,
            tc.tile_pool(name="cpool", bufs=1) as cpool,
            tc.tile_pool(name="opool", bufs=3) as opool,
            tc.tile_pool(name="ofpool", bufs=8) as ofpool,
            tc.tile_pool(name="psum", bufs=8, space="PSUM") as psum_pool,
        ):
            sc_sb = None
            bi_sb = None

            def dequant(eng, ot, ps, nt):
                if eng == "s":
                    nc.scalar.activation(
                        ot,
                        ps,
                        mybir.ActivationFunctionType.Identity,
                        bias=bi_sb[:, nt : nt + 1],
                        scale=sc_sb[:, nt : nt + 1],
                    )
                else:
                    nc.vector.tensor_scalar(
                        ot,
                        ps,
                        sc_sb[:, nt : nt + 1],
                        bi_sb[:, nt : nt + 1],
                        mybir.AluOpType.mult,
                        mybir.AluOpType.add,
                    )

            # ---- DVE first instruction: memset for the warm-up tile.
            warm = cpool.tile([P, MC], dt.bfloat16)
            nc.vector.memset(warm[:], 0.0)

            # ---- startup DMA program: few, big loads, all on the sync ring
            # in need order (the scalar ring is measurably ~2x slower during
            # the 8-core startup crunch; many small early DMAs also collapse
            # ring throughput). Order: w0, x0, w1, then the rest of x.
            w_tiles = {}

            def load_w(eng, nt):
                t = wpool.tile([P, KO, P], dt.bfloat16, tag="w", name=f"w_{nt}")
                eng.dma_start(t[:], wt[nt])
                w_tiles[nt] = t

            load_w(nc.sync, 0)
            x0 = xpool.tile([P, MS], dt.bfloat16, tag="x0", name="x_0")
            nc.sync.dma_start(x0[:], xT[:, 0])
            load_w(nc.sync, 1)
            x_ko = {0: x0}
            for ko in range(1, KO):
                t = xpool.tile([P, MS], dt.bfloat16, tag=f"x{ko}", name=f"x_{ko}")
                nc.sync.dma_start(t[:], xT[:, ko])
                x_ko[ko] = t

            # constants + w2/w3 on sync AFTER the x stream: any concurrent
            # traffic on other rings during the startup crunch delays x and
            # can re-throttle the HAM clock (measured: +1.9us on x1).
            sc_sb = cpool.tile([P, NT], dt.float32)
            nc.sync.dma_start(sc_sb[:], sc)
            bi_sb = cpool.tile([P, NT], dt.float32)
            nc.sync.dma_start(bi_sb[:], bi)
            load_w(nc.sync, 2)
            load_w(nc.sync, 3)

            # ---- warm-up dummies (no DMA deps): PE is HAM-throttled to
            # 1.2 GHz until ~3.4us of sustained activity.
            warm_ps = psum_pool.tile([P, MC], dt.float32, tag="ps", name="warm_ps")
            for _ in range(10):
                nc.tensor.matmul(
                    warm_ps[:], lhsT=warm[:, :P], rhs=warm[:], start=True, stop=True
                )

            # ---- pair phase: nt=0 and nt=1 together, k-outer across all 8
            # PSUM banks: 8 MMs per x k-slice = 1.7us (warm) per 512KB.
            pair_ps = {}
            for nt in (0, 1):
                for ci in range(NMC):
                    pair_ps[(nt, ci)] = psum_pool.tile(
                        [P, MC], dt.float32, tag="ps", name=f"ps_p{nt}_{ci}"
                    )
            for k in range(KO):
                for nt in (0, 1):
                    for ci in range(NMC):
                        off = ci * MC
                        rhs = x_ko[k][:, off : off + MC]
                        lhsT = w_tiles[nt][:, k]
                        nc.tensor.matmul(
                            pair_ps[(nt, ci)][:],
                            lhsT=lhsT,
                            rhs=rhs,
                            start=(k == 0),
                            stop=(k == KO - 1),
                        )
            obig = {}
            for nt in (0, 1):
                obig[nt] = opool.tile([P, MS], dt.float32, tag="o", name=f"o_{nt}")
                for ci in range(NMC):
                    dequant(
                        "s" if ci % 2 == 0 else "v",
                        obig[nt][:, ci * MC : (ci + 1) * MC],
                        pair_ps[(nt, ci)][:],
                        nt,
                    )
            del w_tiles[0], w_tiles[1]
            nc.sync.dma_start(outT_t[0], obig[0][:])
            nc.scalar.dma_start(outT_t[1], obig[1][:])

            def x_rhs(k, off, wd):
                return x_ko[k][:, off : off + wd]

            # ---- nt=2: chunk-outer (bank demand 1 per 1.7us matches the
            # drain rate of the pair phase's 8 banks).
            w_sb = w_tiles.pop(2)
            load_w(nc.sync, 4)
            o2 = opool.tile([P, MS], dt.float32, tag="o", name="o_2")
            for ci in range(NMC):
                ps = psum_pool.tile([P, MC], dt.float32, tag="ps", name=f"ps_2_{ci}")
                for k in range(KO):
                    nc.tensor.matmul(
                        ps[:],
                        lhsT=w_sb[:, k],
                        rhs=x_rhs(k, ci * MC, MC),
                        start=(k == 0),
                        stop=(k == KO - 1),
                    )
                dequant(
                    "s" if ci % 2 == 0 else "v",
                    o2[:, ci * MC : (ci + 1) * MC],
                    ps[:],
                    2,
                )
            nc.sync.dma_start(outT_t[2], o2[:])

            # ---- steady state: nt=3..30, k-outer, weight prefetch distance 2
            # on sync, one batched store per nt on sync.
            for nt in range(3, NT - 1):
                if nt + 2 < NT:
                    load_w(nc.sync, nt + 2)
                w_sb = w_tiles.pop(nt)
                psums = [
                    psum_pool.tile([P, MC], dt.float32, tag="ps", name=f"ps_{nt}_{ci}")
                    for ci in range(NMC)
                ]
                for k in range(KO):
                    for ci in range(NMC):
                        nc.tensor.matmul(
                            psums[ci][:],
                            lhsT=w_sb[:, k],
                            rhs=x_rhs(k, ci * MC, MC),
                            start=(k == 0),
                            stop=(k == KO - 1),
                        )
                ot = opool.tile([P, MS], dt.float32, tag="o", name=f"o_{nt}")
                for ci in range(NMC):
                    dequant(
                        "s" if ci % 2 == 0 else "v",
                        ot[:, ci * MC : (ci + 1) * MC],
                        psums[ci][:],
                        nt,
                    )
                if nt < NT - 2:
                    nc.sync.dma_start(outT_t[nt], ot[:])
                else:
                    # nt=30: store in halves so its 1MB isn't the kernel's
                    # last DMA completion.
                    nc.sync.dma_start(outT_t[nt, :, :1024], ot[:, :1024])
                    nc.sync.dma_start(outT_t[nt, :, 1024:], ot[:, 1024:])

            # ---- tail: nt=31 chunk-outer with narrowing chunks; each bank
            # dequants+stores as soon as its own chain stops; stores spread
            # across both HWDGE rings.
            nt = NT - 1
            w_sb = w_tiles.pop(nt)
            chunks = [
                (0, 512),
                (512, 512),
                (1024, 512),
                (1536, 256),
                (1792, 128),
                (1920, 128),
            ]
            for ci, (off, wd) in enumerate(chunks):
                ps = psum_pool.tile([P, wd], dt.float32, tag="ps", name=f"ps_t_{ci}")
                for k in range(KO):
                    nc.tensor.matmul(
                        ps[:],
                        lhsT=w_sb[:, k],
                        rhs=x_rhs(k, off, wd),
                        start=(k == 0),
                        stop=(k == KO - 1),
                    )
                ot = ofpool.tile([P, MC], dt.float32, tag="of", name=f"of_{ci}")
                ot = ot[:, :wd]
                dequant("s" if ci % 2 == 0 else "v", ot, ps[:], nt)
                if ci % 2 == 1:
                    nc.sync.dma_start(outT_t[nt, :, off : off + wd], ot)
                else:
                    nc.scalar.dma_start(outT_t[nt, :, off : off + wd], ot)

    nc.compile()
    return nc


def _get_nc():
    if "nc" not in _CACHE:
        _CACHE["nc"] = _build()
    return _CACHE["nc"]


def _try_install_ntff_hook():
    """Best-effort: register the axon NTFF profiling hook (the agent image's
    antenv lacks axon_hooks). Returns True if tracing is usable."""
    try:
        import sys
        import types

        import antenv

        if "antenv.axon_hooks" not in sys.modules:
            mod = types.ModuleType("antenv.axon_hooks")
            state = {"hook": None}
            mod.set_axon_ntff_profile_hook = lambda h: state.__setitem__("hook", h)
            mod.get_axon_ntff_profile_hook = lambda: state["hook"]
            sys.modules["antenv.axon_hooks"] = mod
            antenv.axon_hooks = mod

            from trn_agent_boot.trn_boot import _ntff_profile_via_ctypes

            hook = _ntff_profile_via_ctypes("/opt/axon/libaxon_pjrt.so")
            if hook is not None:
                mod.set_axon_ntff_profile_hook(hook)
        return True
    except Exception:
        return False


def kernel(**inputs) -> np.ndarray:
    global LAST_RESULTS
    from concourse.bass_utils import run_bass_kernel_spmd

    x = np.asarray(inputs["x"])
    w = np.asarray(inputs["weight"])
    scale = np.asarray(inputs["scale"], dtype=np.float32)
    bias = np.asarray(inputs["bias"])

    bf16 = ml_dtypes.bfloat16
    nc = _get_nc()

    # weight -> [nt, k_local(part), ko, n_local]
    wt = np.ascontiguousarray(
        w.astype(bf16).reshape(NT, P, KO, P).transpose(0, 3, 2, 1)
    )
    sc = np.ascontiguousarray(scale.reshape(NT, P).T)
    bi = np.ascontiguousarray((bias.astype(np.float32) * scale).reshape(NT, P).T)

    in_maps = []
    for c in range(NCORES):
        xs = x[c * MS : (c + 1) * MS].astype(bf16)  # [MS, K]
        xt = np.ascontiguousarray(xs.T.reshape(KO, P, MS).transpose(1, 0, 2))
        in_maps.append({"xT": xt, "wt": wt, "sc": sc, "bi": bi})

    trace = os.environ.get("BASS_TRACE", "0") == "1" and _try_install_ntff_hook()
    try:
        LAST_RESULTS = run_bass_kernel_spmd(
            nc, in_maps, core_ids=list(range(NCORES)), trace=trace
        )
    except Exception:
        if not trace:
            raise
        # Tracing plumbing is environment-dependent; never let it take down
        # the actual computation.
        os.environ["BASS_NEVER_TRACE"] = "1"
        LAST_RESULTS = run_bass_kernel_spmd(
            nc, in_maps, core_ids=list(range(NCORES)), trace=False
        )

    out = np.empty((M, N), dtype=np.float32)
    for c in range(NCORES):
        out[c * MS : (c + 1) * MS] = LAST_RESULTS.results[c]["outT"].T
    return out
